# revision 1
# baseline (speedup 1.0000x reference)
"""CVFSMN Trainium2 kernel.

Strategy: data-parallel over batch (8 cores, 1 batch row each). Per core:
  out = FSMN_memory(x @ W1 + b1) @ W2 + b2
is decomposed (exactly) as
  out[t] = IDHT( DHT_blocks(x @ W1) * filter ) @ W2  + row + edge(t<39)
using overlap-save block convolution in the Hartley (DHT) domain: the
depthwise 40-tap causal conv diagonalizes into an elementwise spectrum
multiply, so all heavy lifting is dense fp32r matmuls on the PE array.
  - combined taps  w~[l,d] = mw[l+1,d] - mw[0,d]   (l = 0..39)
  - global term    mw[0]*total  ->  rank-1 row via colsum(x) @ W1 .w0. W2
  - bias b1        -> host-precomputed row + 39 edge-correction rows
The colsum is folded into the forward DHT: the transform matrix gets an
extra indicator column (1 for the fresh rows of each overlap-save
window), so spec[:, NF] accumulates the block-local column sum for free
and no second pass over x is needed. Iteration runs in two phases:
  phase 1 (per block): fwd DHT -> mm1 -> pointwise -> Y stored in bf16
  phase 2: rank-1 row, then per block mm2 (bf16) -> inverse DHT -> out
"""
import sys
sys.path.insert(0, "/opt/trn_rl_repo")

import numpy as np
import ml_dtypes

B, T, DIN, DP, DOUT, MEM = 8, 2048, 1024, 1024, 1024, 40
NF, HOP, NB = 384, 345, 6
P = 128
NCORES = 8

_cache = {}


def _host_precompute(W1, b1, W2, b2, mw):
    f64 = np.float64
    W1_, W2_, b1_, b2_, mw_ = (np.asarray(a, f64) for a in (W1, W2, b1, b2, mw))
    w0 = mw_[0]
    wt = mw_[1:MEM + 1] - w0[None, :]            # [40, D]
    tt = np.arange(NF)
    ang = 2 * np.pi * np.outer(tt, tt) / NF
    CAS = np.cos(ang) + np.sin(ang)              # symmetric [t, f]
    # two extra columns: colsum indicator (fresh window rows), zero pad so
    # the fp32r moving free dim stays even (ISA restriction)
    Rfwd = np.concatenate(
        [CAS, (tt >= MEM - 1).astype(f64)[:, None],
         np.zeros((NF, 1), f64)], axis=1)       # [t, f | colsum | pad]
    Rinv = CAS / NF
    hpad = np.zeros((NF, DP), f64)
    hpad[:MEM] = wt
    H = CAS @ hpad                               # [f, d]
    Hrev = np.roll(H[::-1], 1, axis=0)           # H[(N-f)%N]
    Hep = (H + Hrev) / 2
    Ho = (H - Hrev) / 2
    Hep[0] = H[0]
    Hep_aug = np.concatenate(
        [Hep, np.ones((1, DP), f64), np.zeros((1, DP), f64)],
        axis=0)                                  # [f+2, d], row NF = 1.0
    Kt = np.cumsum(wt, axis=0)
    K39 = Kt[-1]
    r_const = (T * (w0 * b1_) + b1_ * K39) @ W2_ + b2_
    E = ((Kt[:39] - K39[None, :]) * b1_[None, :]) @ W2_

    def tile3(a, nt, dt=np.float32):  # [nt*128, F] -> [128, nt, F]
        return np.ascontiguousarray(
            a.reshape(nt, P, -1).transpose(1, 0, 2).astype(dt))

    bf16 = ml_dtypes.bfloat16
    return {
        "W1h": tile3(np.asarray(W1, np.float32), DIN // P),
        "W2h": tile3(np.asarray(W2_, f64), DP // P, bf16),
        "Rfwdh": tile3(Rfwd.astype(np.float32), NF // P),       # [128,3,385]
        "Rinvh": tile3(Rinv[:, NF - HOP:].astype(np.float32), NF // P),
        "Heph": tile3(np.ascontiguousarray(Hep_aug.T).astype(np.float32),
                      DP // P),                                  # [128,8,385]
        "Hoh": tile3(np.ascontiguousarray(Ho.T).astype(np.float32), DP // P),
        "w0h": np.ascontiguousarray(
            w0.reshape(DP // P, P).T.astype(np.float32)),        # [128, 8]
        "rconst_rep": np.broadcast_to(
            r_const.astype(np.float32), (P, DOUT)).copy(),
        "Eh": E.astype(np.float32),
        "ztile": np.zeros((P, DIN), np.float32),
    }


def _build_nc(repeat=1, ablate=()):
    import concourse.mybir as mybir
    import concourse.tile as tile
    from concourse import bacc

    dt = mybir.dt
    AL = mybir.AluOpType
    f32, f32r, bf16 = dt.float32, dt.float32r, dt.bfloat16
    KD, KJ, KF, KT = DIN // P, DP // P, NF // P, T // P
    NF1 = NF + 2

    nc = bacc.Bacc(None, target_bir_lowering=False)
    x_d = nc.declare_dram_parameter("x", [T, DIN], f32r, isOutput=False)
    W1_d = nc.declare_dram_parameter("W1h", [P, KD, DP], f32r, isOutput=False)
    W2_d = nc.declare_dram_parameter("W2h", [P, KJ, DOUT], bf16, isOutput=False)
    Rf_d = nc.declare_dram_parameter("Rfwdh", [P, KF, NF1], f32r, isOutput=False)
    Ri_d = nc.declare_dram_parameter("Rinvh", [P, KF, HOP], f32r, isOutput=False)
    Hep_d = nc.declare_dram_parameter("Heph", [P, KJ, NF1], f32, isOutput=False)
    Ho_d = nc.declare_dram_parameter("Hoh", [P, KJ, NF], f32, isOutput=False)
    w0_d = nc.declare_dram_parameter("w0h", [P, KJ], f32, isOutput=False)
    rc_d = nc.declare_dram_parameter("rconst_rep", [P, DOUT], f32, isOutput=False)
    E_d = nc.declare_dram_parameter("Eh", [39, DOUT], f32, isOutput=False)
    zt_d = nc.declare_dram_parameter("ztile", [P, DIN], f32r, isOutput=False)
    out_d = nc.declare_dram_parameter("out", [T, DOUT], f32, isOutput=True)

    with tile.TileContext(nc) as tc:
        with (
            tc.tile_pool(name="wres", bufs=1) as wres,
            tc.tile_pool(name="xpool", bufs=6) as xpool,
            tc.tile_pool(name="xs", bufs=9) as xs_pool,
            tc.tile_pool(name="tmp", bufs=3) as tmp_pool,
            tc.tile_pool(name="zpool", bufs=6) as z_pool,
            tc.tile_pool(name="opool", bufs=3) as out_pool,
            tc.tile_pool(name="psA", bufs=2, space="PSUM") as psA,
            tc.tile_pool(name="psB", bufs=2, space="PSUM") as psB,
            tc.tile_pool(name="psC", bufs=2, space="PSUM") as psC,
            tc.tile_pool(name="psD", bufs=2, space="PSUM") as psD,
        ):
            # Resident tensors.
            Rf_sb = wres.tile([P, KF, NF1], f32r)
            W1_sb = wres.tile([P, KD, DP], f32r)
            Hep_sb = wres.tile([P, KJ, NF1], f32)
            Ho_sb = wres.tile([P, KJ, NF], f32)
            W2_sb = wres.tile([P, KJ, DOUT], bf16)
            Ri_sb = wres.tile([P, KF, HOP], f32r)
            w0_sb = wres.tile([P, KJ], f32)
            rc_sb = wres.tile([P, DOUT], f32)
            E_sb = wres.tile([39, DOUT], f32)
            zt_sb = wres.tile([P, DIN], f32r)
            row_sb = wres.tile([P, DOUT], f32)    # r_const + rank-1 row
            Y_sb = wres.tile([P, NB, KJ, NF], bf16)
            t2a_sb = wres.tile([P, KJ], f32)      # colsum @ W1 accumulator
            t2b_sb = wres.tile([P, KJ], bf16)

            def load_weights_early():
                for kc in range(KF):
                    nc.sync.dma_start(Rf_sb[:, kc], Rf_d[:, kc])

            def load_weights_mid():
                for dc in range(KD):
                    q = nc.scalar if dc % 2 == 0 else nc.gpsimd
                    q.dma_start(W1_sb[:, dc], W1_d[:, dc])
                for jt in range(2):
                    nc.scalar.dma_start(Hep_sb[:, jt], Hep_d[:, jt])
                    nc.scalar.dma_start(Ho_sb[:, jt], Ho_d[:, jt])

            def load_weights_late():
                for jt in range(2, KJ):
                    nc.scalar.dma_start(Hep_sb[:, jt], Hep_d[:, jt])
                    nc.scalar.dma_start(Ho_sb[:, jt], Ho_d[:, jt])
                nc.sync.dma_start(w0_sb[:], w0_d[:])
                for jc in range(KJ):
                    nc.gpsimd.dma_start(W2_sb[:, jc], W2_d[:, jc])
                nc.scalar.dma_start(Ri_sb[:], Ri_d[:])
                nc.sync.dma_start(rc_sb[:], rc_d[:])
                nc.sync.dma_start(E_sb[:], E_d[:])
                nc.gpsimd.dma_start(zt_sb[:], zt_d[:])

            def load_xw(b):
                if "noxdma" in ablate:
                    return [zt_sb] * KF
                xw = []
                for kc in range(KF):
                    row0 = HOP * b - (NF - HOP) + P * kc
                    lo, hi = max(row0, 0), min(row0 + P, T)
                    if hi <= lo:
                        xw.append(zt_sb)
                        continue
                    q = (nc.sync, nc.scalar, nc.gpsimd)[kc]
                    xt = xpool.tile([P, DIN], f32r, tag="xt")
                    if lo > row0:
                        q.dma_start(xt[0:lo - row0, :],
                                    zt_d[0:lo - row0, :])
                    q.dma_start(xt[lo - row0:hi - row0, :], x_d[lo:hi, :])
                    if hi < row0 + P:
                        q.dma_start(xt[hi - row0:P, :],
                                    zt_d[hi - row0:P, :])
                    xw.append(xt)
                return xw

            def fwd_stage(xw):
                # spec[dchunk, f 0..383 | colsum 384]
                xs = []
                for dtl in range(KD):
                    t = xs_pool.tile([P, NF1], f32r, tag="xs")
                    pst = psA.tile([P, NF1], f32, tag="a")
                    for kc in range(KF):
                        nc.tensor.matmul(
                            pst[:], xw[kc][:, dtl * P:(dtl + 1) * P],
                            Rf_sb[:, kc], start=(kc == 0),
                            stop=(kc == KF - 1))
                    nc.scalar.copy(t[:], pst[:])
                    xs.append(t)
                return xs

            def mm1_pw_stage(b, xs):
                for jt in range(KJ):
                    pst = psB.tile([P, NF1], f32, tag="b")
                    for dc in range(KD):
                        nc.tensor.matmul(
                            pst[:], W1_sb[:, dc, jt * P:(jt + 1) * P],
                            xs[dc][:], start=(dc == 0), stop=(dc == KD - 1))
                    if "pw1" in ablate:
                        nc.gpsimd.tensor_copy(Y_sb[:, b, jt, :], pst[:, :NF])
                        if b == 0:
                            nc.vector.tensor_copy(
                                t2a_sb[:, jt:jt + 1], pst[:, NF:NF + 1])
                        continue
                    tmp = tmp_pool.tile([P, NF], f32, tag="pw")
                    tmp2 = tmp_pool.tile([P, NF1], f32, tag="pw2")
                    nc.vector.tensor_tensor(
                        tmp[:, 1:], pst[:, NF - 1:0:-1],
                        Ho_sb[:, jt, 1:], AL.mult)
                    nc.vector.tensor_tensor(
                        tmp2[:], pst[:], Hep_sb[:, jt, :], AL.mult)
                    nc.gpsimd.tensor_tensor(
                        Y_sb[:, b, jt, 1:], tmp2[:, 1:NF], tmp[:, 1:], AL.add)
                    nc.scalar.copy(Y_sb[:, b, jt, 0:1], tmp2[:, 0:1])
                    # colsum@W1 partial lives in col NF (Hep row NF == 1)
                    if b == 0:
                        nc.gpsimd.tensor_copy(
                            t2a_sb[:, jt:jt + 1], tmp2[:, NF:NF + 1])
                    else:
                        nc.gpsimd.tensor_tensor(
                            t2a_sb[:, jt:jt + 1], t2a_sb[:, jt:jt + 1],
                            tmp2[:, NF:NF + 1], AL.add)

            def row_stage():
                nc.vector.tensor_tensor(t2b_sb[:], t2a_sb[:], w0_sb[:],
                                        AL.mult)
                for ntl in range(2):
                    rr_ps = psC.tile([P, 512], f32, tag="c")
                    for jc in range(KJ):
                        nc.tensor.matmul(
                            rr_ps[:],
                            t2b_sb[:, jc:jc + 1].to_broadcast((P, P)),
                            W2_sb[:, jc, ntl * 512:(ntl + 1) * 512],
                            start=(jc == 0), stop=(jc == KJ - 1))
                    nc.vector.tensor_tensor(
                        row_sb[:, ntl * 512:(ntl + 1) * 512], rr_ps[:],
                        rc_sb[:, ntl * 512:(ntl + 1) * 512], AL.add)

            def mm2_part(b):
                zs = {}
                for ntl in range(2):
                    for ft in range(KF):
                        pst = psC.tile([P, 512], f32, tag="c")
                        for jc in range(KJ):
                            nc.tensor.matmul(
                                pst[:],
                                Y_sb[:, b, jc, ft * P:(ft + 1) * P],
                                W2_sb[:, jc, ntl * 512:(ntl + 1) * 512],
                                start=(jc == 0), stop=(jc == KJ - 1))
                        zt = z_pool.tile([P, 512], f32r, tag="z")
                        nc.scalar.copy(zt[:], pst[:])
                        zs[(ntl, ft)] = zt
                return zs

            def inv_part(b, zs):
                V = min(HOP, T - HOP * b)
                tts = []
                off = 0
                while off < V:
                    tts.append((off, min(P, V - off)))
                    off += P
                for ti, (off, ln) in enumerate(tts):
                    ot = out_pool.tile([P, DOUT], f32, tag="o")
                    for ntl in range(2):
                        pst = psD.tile([P, 512], f32, tag="d")
                        for fc in range(KF):
                            nc.tensor.matmul(
                                pst[:ln], Ri_sb[:, fc, off:off + ln],
                                zs[(ntl, fc)][:], start=(fc == 0),
                                stop=(fc == KF - 1))
                        nc.vector.tensor_tensor(
                            ot[:ln, ntl * 512:(ntl + 1) * 512], pst[:ln],
                            row_sb[:ln, ntl * 512:(ntl + 1) * 512], AL.add)
                        if b == 0 and off == 0:
                            nc.vector.tensor_tensor(
                                ot[:39, ntl * 512:(ntl + 1) * 512],
                                ot[:39, ntl * 512:(ntl + 1) * 512],
                                E_sb[:, ntl * 512:(ntl + 1) * 512], AL.add)
                    if "nooutdma" in ablate:
                        continue
                    r0 = HOP * b + off
                    q = (nc.sync, nc.gpsimd, nc.scalar)[(b + ti) % 3]
                    q.dma_start(out_d[r0:r0 + ln, :], ot[:ln, :])

            def phase2():
                # row matmuls slot between mm2(0) and inv(0): they hide the
                # pointwise tail (t2 accumulation) and block 0's z copies
                zs = mm2_part(0)
                row_stage()
                inv_part(0, zs)
                for b in range(1, NB):
                    zs = mm2_part(b)
                    inv_part(b, zs)

            def body():
                for b in range(NB):
                    xw = load_xw(b)
                    xs = fwd_stage(xw)
                    mm1_pw_stage(b, xs)
                phase2()

            if repeat == 1:
                load_weights_early()
                xw0 = load_xw(0)
                load_weights_mid()
                xs0 = fwd_stage(xw0)
                load_weights_late()
                mm1_pw_stage(0, xs0)
                for b in range(1, NB):
                    xw = load_xw(b)
                    xs = fwd_stage(xw)
                    mm1_pw_stage(b, xs)
                phase2()
            else:
                load_weights_early()
                load_weights_mid()
                load_weights_late()
                with tc.For_i(0, repeat, 1):
                    body()
    nc.compile()
    return nc


def _get_nc(repeat=1, ablate=()):
    key = ("nc", repeat, tuple(ablate))
    if key not in _cache:
        _cache[key] = _build_nc(repeat, ablate)
    return _cache[key]


def _in_maps(inputs):
    key = "pc"
    if key not in _cache:
        _cache[key] = _host_precompute(
            inputs["W1"], inputs["bias1"], inputs["W2"], inputs["bias2"],
            inputs["memory_weights"])
    pc = _cache[key]
    x = np.ascontiguousarray(np.asarray(inputs["input_data"], np.float32))
    maps = []
    for c in range(NCORES):
        m = {"x": x[c]}
        m.update(pc)
        maps.append(m)
    return maps


def kernel(**inputs):
    from concourse.bass_utils import run_bass_kernel_spmd
    nc = _get_nc(repeat=1)
    maps = _in_maps(inputs)
    res = run_bass_kernel_spmd(nc, maps, list(range(NCORES)))
    out = np.stack([res.results[c]["out"] for c in range(NCORES)], axis=0)
    return out.astype(np.float32)



# revision 14
# speedup vs baseline: 1.8455x; 1.8455x over previous
"""CVFSMN Trainium2 kernel.

Strategy: data-parallel over batch (8 cores, 1 batch row each). Per core:
  out = FSMN_memory(x @ W1 + b1) @ W2 + b2
is decomposed (exactly) as
  out[t] = IDHT( DHT_blocks(x @ W1) * filter ) @ W2  + row + edge(t<39)
using overlap-save block convolution in the Hartley (DHT) domain: the
depthwise 40-tap causal conv diagonalizes into an elementwise spectrum
multiply, so all heavy lifting is dense fp32r matmuls on the PE array.
  - combined taps  w~[l,d] = mw[l+1,d] - mw[0,d]   (l = 0..39)
  - global term    mw[0]*total  ->  rank-1 row via colsum(x) @ W1 .w0. W2
  - bias b1        -> host-precomputed row + 39 edge-correction rows
The colsum is folded into the forward DHT: the transform matrix gets an
extra indicator column (1 for the fresh rows of each overlap-save
window), so spec[:, NF] accumulates the block-local column sum for free
and no second pass over x is needed. Iteration runs in two phases:
  phase 1 (per block): fwd DHT -> mm1 -> pointwise -> Y stored in bf16
  phase 2: rank-1 row, then per block mm2 (bf16) -> inverse DHT -> out
"""
import sys
sys.path.insert(0, "/opt/trn_rl_repo")

import numpy as np
import ml_dtypes

B, T, DIN, DP, DOUT, MEM = 8, 2048, 1024, 1024, 1024, 40
NF, HOP, NB = 384, 345, 6
P = 128
NCORES = 8

_cache = {}


def _host_precompute(W1, b1, W2, b2, mw):
    f64 = np.float64
    W1_, W2_, b1_, b2_, mw_ = (np.asarray(a, f64) for a in (W1, W2, b1, b2, mw))
    w0 = mw_[0]
    wt = mw_[1:MEM + 1] - w0[None, :]            # [40, D]
    tt = np.arange(NF)
    ang = 2 * np.pi * np.outer(tt, tt) / NF
    CAS = np.cos(ang) + np.sin(ang)              # symmetric [t, f]
    # two extra columns: colsum indicator (fresh window rows), zero pad so
    # the fp32r moving free dim stays even (ISA restriction)
    Rfwd = np.concatenate(
        [CAS, (tt >= MEM - 1).astype(f64)[:, None],
         np.zeros((NF, 1), f64)], axis=1)       # [t, f | colsum | pad]
    Rinv = CAS / NF
    hpad = np.zeros((NF, DP), f64)
    hpad[:MEM] = wt
    H = CAS @ hpad                               # [f, d]
    Hrev = np.roll(H[::-1], 1, axis=0)           # H[(N-f)%N]
    Hep = (H + Hrev) / 2
    Ho = (H - Hrev) / 2
    Hep[0] = H[0]
    Hep_aug = np.concatenate(
        [Hep, np.ones((1, DP), f64), np.zeros((1, DP), f64)],
        axis=0)                                  # [f+2, d], row NF = 1.0
    Kt = np.cumsum(wt, axis=0)
    K39 = Kt[-1]
    r_const = (T * (w0 * b1_) + b1_ * K39) @ W2_ + b2_
    E = ((Kt[:39] - K39[None, :]) * b1_[None, :]) @ W2_

    def tile3(a, nt, dt=np.float32):  # [nt*128, F] -> [128, nt, F]
        return np.ascontiguousarray(
            a.reshape(nt, P, -1).transpose(1, 0, 2).astype(dt))

    bf16 = ml_dtypes.bfloat16
    return {
        "W1h": tile3(np.asarray(W1, np.float32), DIN // P, bf16),
        "W2h": tile3(np.asarray(W2_, f64), DP // P, bf16),
        "Rfwdh": tile3(Rfwd, NF // P, bf16),                    # [128,3,386]
        "Rinvh": tile3(Rinv[:, NF - HOP:], NF // P, bf16),
        "Heph": tile3(np.ascontiguousarray(Hep_aug.T), DP // P, bf16),
        "Hoh": tile3(np.ascontiguousarray(Ho.T), DP // P, bf16),
        "w0h": np.ascontiguousarray(
            w0.reshape(DP // P, P).T.astype(np.float32)),        # [128, 8]
        "rconst_rep": np.broadcast_to(
            r_const.astype(np.float32), (P, DOUT)).copy(),
        "Eh": E.astype(np.float32),
        "ztile": np.zeros((P, DIN), np.float32),
    }


def _build_nc(repeat=1, ablate=()):
    import concourse.mybir as mybir
    import concourse.tile as tile
    from concourse import bacc

    dt = mybir.dt
    AL = mybir.AluOpType
    f32, f32r, bf16 = dt.float32, dt.float32r, dt.bfloat16
    KD, KJ, KF, KT = DIN // P, DP // P, NF // P, T // P
    NF1 = NF + 2

    nc = bacc.Bacc(None, target_bir_lowering=False)
    x_d = nc.declare_dram_parameter("x", [T, DIN], f32, isOutput=False)
    W1_d = nc.declare_dram_parameter("W1h", [P, KD, DP], bf16, isOutput=False)
    W2_d = nc.declare_dram_parameter("W2h", [P, KJ, DOUT], bf16, isOutput=False)
    Rf_d = nc.declare_dram_parameter("Rfwdh", [P, KF, NF1], bf16, isOutput=False)
    Ri_d = nc.declare_dram_parameter("Rinvh", [P, KF, HOP], bf16, isOutput=False)
    Hep_d = nc.declare_dram_parameter("Heph", [P, KJ, NF1], bf16, isOutput=False)
    Ho_d = nc.declare_dram_parameter("Hoh", [P, KJ, NF], bf16, isOutput=False)
    w0_d = nc.declare_dram_parameter("w0h", [P, KJ], f32, isOutput=False)
    rc_d = nc.declare_dram_parameter("rconst_rep", [P, DOUT], f32, isOutput=False)
    E_d = nc.declare_dram_parameter("Eh", [39, DOUT], f32, isOutput=False)
    zt_d = nc.declare_dram_parameter("ztile", [P, DIN], f32, isOutput=False)
    out_d = nc.declare_dram_parameter("out", [T, DOUT], f32, isOutput=True)

    with tile.TileContext(nc) as tc:
        with (
            tc.tile_pool(name="wres", bufs=1) as wres,
            tc.tile_pool(name="xpool", bufs=6) as xpool,
            tc.tile_pool(name="xbpool", bufs=6) as xbpool,
            tc.tile_pool(name="xs", bufs=9) as xs_pool,
            tc.tile_pool(name="tmp", bufs=3) as tmp_pool,
            tc.tile_pool(name="zpool", bufs=6) as z_pool,
            tc.tile_pool(name="opool", bufs=3) as out_pool,
            tc.tile_pool(name="psA", bufs=2, space="PSUM") as psA,
            tc.tile_pool(name="psB", bufs=2, space="PSUM") as psB,
            tc.tile_pool(name="psC", bufs=2, space="PSUM") as psC,
            tc.tile_pool(name="psD", bufs=2, space="PSUM") as psD,
        ):
            # Resident tensors.
            Rf_sb = wres.tile([P, KF, NF1], bf16)
            W1_sb = wres.tile([P, KD, DP], bf16)
            Hep_sb = wres.tile([P, KJ, NF1], bf16)
            Ho_sb = wres.tile([P, KJ, NF], bf16)
            W2_sb = wres.tile([P, KJ, DOUT], bf16)
            Ri_sb = wres.tile([P, KF, HOP], bf16)
            w0_sb = wres.tile([P, KJ], f32)
            rc_sb = wres.tile([P, DOUT], f32)
            E_sb = wres.tile([39, DOUT], f32)
            zt_sb = wres.tile([P, DIN], f32)
            ztb_sb = wres.tile([P, DIN], bf16)
            row_sb = wres.tile([P, DOUT], f32)    # r_const + rank-1 row
            Y_sb = wres.tile([P, NB, KJ, NF], bf16)
            t2a_sb = wres.tile([P, KJ], f32)      # colsum @ W1 accumulator
            t2b_sb = wres.tile([P, KJ], bf16)

            def load_weights_early():
                for kc in range(KF):
                    nc.sync.dma_start(Rf_sb[:, kc], Rf_d[:, kc])

            def load_weights_mid():
                for dc in range(KD):
                    q = nc.scalar if dc % 2 == 0 else nc.gpsimd
                    q.dma_start(W1_sb[:, dc], W1_d[:, dc])
                for jt in range(2):
                    nc.scalar.dma_start(Hep_sb[:, jt], Hep_d[:, jt])
                    nc.scalar.dma_start(Ho_sb[:, jt], Ho_d[:, jt])

            def load_weights_late():
                for jt in range(2, KJ):
                    nc.scalar.dma_start(Hep_sb[:, jt], Hep_d[:, jt])
                    nc.scalar.dma_start(Ho_sb[:, jt], Ho_d[:, jt])
                nc.sync.dma_start(w0_sb[:], w0_d[:])
                for jc in range(KJ):
                    nc.gpsimd.dma_start(W2_sb[:, jc], W2_d[:, jc])
                nc.scalar.dma_start(Ri_sb[:], Ri_d[:])
                nc.sync.dma_start(rc_sb[:], rc_d[:])
                nc.sync.dma_start(E_sb[:], E_d[:])
                nc.gpsimd.dma_start(zt_sb[:], zt_d[:])
                nc.gpsimd.tensor_copy(ztb_sb[:], zt_sb[:])

            def load_xw(b):
                if "noxdma" in ablate:
                    return [ztb_sb] * KF
                xw = []
                for kc in range(KF):
                    row0 = HOP * b - (NF - HOP) + P * kc
                    lo, hi = max(row0, 0), min(row0 + P, T)
                    if hi <= lo:
                        xw.append(ztb_sb)
                        continue
                    q = (nc.sync, nc.scalar, nc.gpsimd)[kc]
                    xt = xpool.tile([P, DIN], f32, tag="xt")
                    if lo > row0:
                        q.dma_start(xt[0:lo - row0, :],
                                    zt_d[0:lo - row0, :])
                    q.dma_start(xt[lo - row0:hi - row0, :], x_d[lo:hi, :])
                    if hi < row0 + P:
                        q.dma_start(xt[hi - row0:P, :],
                                    zt_d[hi - row0:P, :])
                    xb = xbpool.tile([P, DIN], bf16, tag="xb")
                    nc.gpsimd.tensor_copy(xb[:], xt[:])
                    xw.append(xb)
                return xw

            def fwd_stage(xw):
                # spec[dchunk, f 0..383 | colsum 384]
                xs = []
                for dtl in range(KD):
                    t = xs_pool.tile([P, NF1], bf16, tag="xs")
                    pst = psA.tile([P, NF1], f32, tag="a")
                    for kc in range(KF):
                        nc.tensor.matmul(
                            pst[:], xw[kc][:, dtl * P:(dtl + 1) * P],
                            Rf_sb[:, kc], start=(kc == 0),
                            stop=(kc == KF - 1))
                    nc.scalar.copy(t[:], pst[:])
                    xs.append(t)
                return xs

            def mm1_pw_stage(b, xs):
                for jt in range(KJ):
                    pst = psB.tile([P, NF1], f32, tag="b")
                    for dc in range(KD):
                        nc.tensor.matmul(
                            pst[:], W1_sb[:, dc, jt * P:(jt + 1) * P],
                            xs[dc][:], start=(dc == 0), stop=(dc == KD - 1))
                    if "pw1" in ablate:
                        nc.gpsimd.tensor_copy(Y_sb[:, b, jt, :], pst[:, :NF])
                        if b == 0:
                            nc.vector.tensor_copy(
                                t2a_sb[:, jt:jt + 1], pst[:, NF:NF + 1])
                        continue
                    tmp = tmp_pool.tile([P, NF], f32, tag="pw")
                    tmp2 = tmp_pool.tile([P, NF1], f32, tag="pw2")
                    nc.vector.tensor_tensor(
                        tmp[:, 1:], pst[:, NF - 1:0:-1],
                        Ho_sb[:, jt, 1:], AL.mult)
                    nc.vector.tensor_tensor(
                        tmp2[:], pst[:], Hep_sb[:, jt, :], AL.mult)
                    nc.gpsimd.tensor_tensor(
                        Y_sb[:, b, jt, 1:], tmp2[:, 1:NF], tmp[:, 1:], AL.add)
                    nc.scalar.copy(Y_sb[:, b, jt, 0:1], tmp2[:, 0:1])
                    # colsum@W1 partial lives in col NF (Hep row NF == 1)
                    if b == 0:
                        nc.gpsimd.tensor_copy(
                            t2a_sb[:, jt:jt + 1], tmp2[:, NF:NF + 1])
                    else:
                        nc.gpsimd.tensor_tensor(
                            t2a_sb[:, jt:jt + 1], t2a_sb[:, jt:jt + 1],
                            tmp2[:, NF:NF + 1], AL.add)

            def row_stage():
                nc.vector.tensor_tensor(t2b_sb[:], t2a_sb[:], w0_sb[:],
                                        AL.mult)
                for ntl in range(2):
                    rr_ps = psC.tile([P, 512], f32, tag="c")
                    for jc in range(KJ):
                        nc.tensor.matmul(
                            rr_ps[:],
                            t2b_sb[:, jc:jc + 1].to_broadcast((P, P)),
                            W2_sb[:, jc, ntl * 512:(ntl + 1) * 512],
                            start=(jc == 0), stop=(jc == KJ - 1))
                    nc.vector.tensor_tensor(
                        row_sb[:, ntl * 512:(ntl + 1) * 512], rr_ps[:],
                        rc_sb[:, ntl * 512:(ntl + 1) * 512], AL.add)

            def mm2_part(b):
                zs = {}
                for ntl in range(2):
                    for ft in range(KF):
                        pst = psC.tile([P, 512], f32, tag="c")
                        for jc in range(KJ):
                            nc.tensor.matmul(
                                pst[:],
                                Y_sb[:, b, jc, ft * P:(ft + 1) * P],
                                W2_sb[:, jc, ntl * 512:(ntl + 1) * 512],
                                start=(jc == 0), stop=(jc == KJ - 1))
                        zt = z_pool.tile([P, 512], bf16, tag="z")
                        nc.scalar.copy(zt[:], pst[:])
                        zs[(ntl, ft)] = zt
                return zs

            def inv_part(b, zs):
                V = min(HOP, T - HOP * b)
                tts = []
                off = 0
                while off < V:
                    tts.append((off, min(P, V - off)))
                    off += P
                for ti, (off, ln) in enumerate(tts):
                    ot = out_pool.tile([P, DOUT], f32, tag="o")
                    for ntl in range(2):
                        pst = psD.tile([P, 512], f32, tag="d")
                        for fc in range(KF):
                            nc.tensor.matmul(
                                pst[:ln], Ri_sb[:, fc, off:off + ln],
                                zs[(ntl, fc)][:], start=(fc == 0),
                                stop=(fc == KF - 1))
                        nc.vector.tensor_tensor(
                            ot[:ln, ntl * 512:(ntl + 1) * 512], pst[:ln],
                            row_sb[:ln, ntl * 512:(ntl + 1) * 512], AL.add)
                        if b == 0 and off == 0:
                            nc.vector.tensor_tensor(
                                ot[:39, ntl * 512:(ntl + 1) * 512],
                                ot[:39, ntl * 512:(ntl + 1) * 512],
                                E_sb[:, ntl * 512:(ntl + 1) * 512], AL.add)
                    if "nooutdma" in ablate:
                        continue
                    r0 = HOP * b + off
                    q = (nc.sync, nc.gpsimd, nc.scalar)[(b + ti) % 3]
                    q.dma_start(out_d[r0:r0 + ln, :], ot[:ln, :])

            def phase2():
                # row matmuls slot between mm2(0) and inv(0): they hide the
                # pointwise tail (t2 accumulation) and block 0's z copies
                zs = mm2_part(0)
                row_stage()
                inv_part(0, zs)
                for b in range(1, NB):
                    zs = mm2_part(b)
                    inv_part(b, zs)

            def body():
                for b in range(NB):
                    xw = load_xw(b)
                    xs = fwd_stage(xw)
                    mm1_pw_stage(b, xs)
                phase2()

            if repeat == 1:
                load_weights_early()
                xw0 = load_xw(0)
                load_weights_mid()
                xs0 = fwd_stage(xw0)
                load_weights_late()
                mm1_pw_stage(0, xs0)
                for b in range(1, NB):
                    xw = load_xw(b)
                    xs = fwd_stage(xw)
                    mm1_pw_stage(b, xs)
                phase2()
            else:
                load_weights_early()
                load_weights_mid()
                load_weights_late()
                with tc.For_i(0, repeat, 1):
                    body()
    nc.compile()
    return nc


def _get_nc(repeat=1, ablate=()):
    key = ("nc", repeat, tuple(ablate))
    if key not in _cache:
        _cache[key] = _build_nc(repeat, ablate)
    return _cache[key]


def _in_maps(inputs):
    key = "pc"
    if key not in _cache:
        _cache[key] = _host_precompute(
            inputs["W1"], inputs["bias1"], inputs["W2"], inputs["bias2"],
            inputs["memory_weights"])
    pc = _cache[key]
    x = np.ascontiguousarray(np.asarray(inputs["input_data"], np.float32))
    maps = []
    for c in range(NCORES):
        m = {"x": x[c]}
        m.update(pc)
        maps.append(m)
    return maps


def kernel(**inputs):
    from concourse.bass_utils import run_bass_kernel_spmd
    nc = _get_nc(repeat=1)
    maps = _in_maps(inputs)
    res = run_bass_kernel_spmd(nc, maps, list(range(NCORES)))
    out = np.stack([res.results[c]["out"] for c in range(NCORES)], axis=0)
    return out.astype(np.float32)

